# revision 1
# baseline (speedup 1.0000x reference)
"""Trainium2 Bass kernel for nn_BatchDropTop (topk row masking).

Reference math: per sample b, act = sum_c x[b,c,:,:]^2  -> [H,W]; L2-normalize
over flattened (H,W) (a positive per-sample scale -- cannot change any
ordering, so it is skipped); row score = max_w act -> [H]; drop (zero) the
rh=8 rows with the largest score; out = x * row_mask.

Kernel strategy (pure data parallel, batch 64 -> 8 samples on each of 8
cores; per core, per sample):
  - DMA x[s] (2048x24x8 f32, 1.5 MB) into SBUF as [128p, 16k, 192hw]
    (partition p holds channels 16p..16p+15; contiguous 12KB per partition).
  - ACT: square elementwise.
  - PE: 16 accumulating matmuls with a ones[128,1] stationary vector reduce
    the channel dim -> act [1, 192] in PSUM.
  - DVE: rowmax[1,24] = max over w; top8 = vector.max (8 largest, desc);
    mask[1,24] = (rowmax < top8[7]) as 1.0/0.0.  (Exactly the top-8 rows
    get 0; validated tie-free on the real inputs with 4.4e-5 min rel gap.)
  - DVE copy expands mask over w -> [1,192]; PE ones[1,128] matmul
    broadcasts it to [128,192] PSUM.
  - DVE: y = x * mask (mask AP broadcast over the 16 chunk dim), DMA out.

Everything is read from HBM once and written once: 25.2 MB per core
~= 70 us at the ~358 GB/s per-core HBM roofline; ACT/PE/DVE each have
~20-30 us of work, overlapped by the Tile scheduler.
"""

import sys

import numpy as np

for _p in ("/opt/trn_rl_repo", "/root/.axon_site/_ro/trn_rl_repo"):
    if _p not in sys.path:
        sys.path.append(_p)

B, C, H, W = 64, 2048, 24, 8
N_CORES = 8
BS = B // N_CORES  # samples per core
P = 128            # SBUF partitions
KC = C // P        # channel chunks per sample
HW = H * W
RH = 8             # rows to drop == round(0.33 * 24)

_cache = {}


def _build_nc():
    from concourse import bacc, mybir, tile

    f32 = mybir.dt.float32
    nc = bacc.Bacc("TRN2", target_bir_lowering=False, debug=False,
                   num_devices=N_CORES)
    x_in = nc.dram_tensor("x", [BS, C, H, W], f32, kind="ExternalInput")
    y_out = nc.dram_tensor("out", [BS, C, H, W], f32, kind="ExternalOutput")

    with tile.TileContext(nc) as tc:
        with (
            tc.tile_pool(name="xp", bufs=3) as xp,
            tc.tile_pool(name="sq", bufs=2) as sqp,
            tc.tile_pool(name="yp", bufs=2) as yp,
            tc.tile_pool(name="const", bufs=1) as constp,
            tc.tile_pool(name="small", bufs=BS) as smallp,
            tc.tile_pool(name="psA", bufs=3, space="PSUM") as psA,
            tc.tile_pool(name="psB", bufs=3, space="PSUM") as psB,
        ):
            ones_col = constp.tile([P, 1], f32)  # stationary K=128 reducer
            nc.vector.memset(ones_col[:], 1.0)
            ones_row = constp.tile([1, P], f32)  # stationary K=1 broadcaster
            nc.vector.memset(ones_row[:], 1.0)

            for s in range(BS):
                xt = xp.tile([P, KC, HW], f32, tag="x")
                nc.sync.dma_start(
                    out=xt[:],
                    in_=x_in[s].rearrange("(p k) h w -> p k (h w)", p=P),
                )

                xsq = sqp.tile([P, KC, HW], f32, tag="sq")
                nc.scalar.square(xsq[:], xt[:])

                act = psA.tile([1, HW], f32, tag="act")
                for k in range(KC):
                    nc.tensor.matmul(
                        act[:], ones_col[:], xsq[:, k, :],
                        start=(k == 0), stop=(k == KC - 1),
                    )

                rowmax = smallp.tile([1, H], f32, tag="rowmax")
                nc.vector.tensor_reduce(
                    rowmax[:],
                    act[:].rearrange("p (h w) -> p h w", h=H),
                    axis=mybir.AxisListType.X,
                    op=mybir.AluOpType.max,
                )
                top8 = smallp.tile([1, RH], f32, tag="top8")
                nc.vector.max(top8[:], rowmax[:])
                maskh = smallp.tile([1, H], f32, tag="maskh")
                nc.vector.tensor_single_scalar(
                    maskh[:], rowmax[:], top8[0:1, RH - 1:RH],
                    mybir.AluOpType.is_lt,
                )
                maskhw = smallp.tile([1, HW], f32, tag="maskhw")
                nc.vector.tensor_copy(
                    maskhw[:].rearrange("p (h w) -> p h w", h=H),
                    maskh[:].unsqueeze(2).broadcast_to([1, H, W]),
                )

                mb = psB.tile([P, HW], f32, tag="mb")
                nc.tensor.matmul(mb[:], ones_row[:], maskhw[:],
                                 start=True, stop=True)

                yt = yp.tile([P, KC, HW], f32, tag="y")
                nc.vector.tensor_tensor(
                    yt[:], xt[:],
                    mb[:].unsqueeze(1).broadcast_to([P, KC, HW]),
                    op=mybir.AluOpType.mult,
                )
                nc.sync.dma_start(
                    out=y_out[s].rearrange("(p k) h w -> p k (h w)", p=P),
                    in_=yt[:],
                )

    nc.compile()
    return nc


def get_nc():
    if "nc" not in _cache:
        _cache["nc"] = _build_nc()
    return _cache["nc"]


def kernel(x):
    from concourse.bass_utils import run_bass_kernel_spmd

    x = np.ascontiguousarray(np.asarray(x, dtype=np.float32))
    assert x.shape == (B, C, H, W), x.shape
    nc = get_nc()
    in_maps = [{"x": x[i * BS:(i + 1) * BS]} for i in range(N_CORES)]
    res = run_bass_kernel_spmd(nc, in_maps, list(range(N_CORES)))
    return np.concatenate(
        [res.results[i]["out"] for i in range(N_CORES)], axis=0
    )


# revision 3
# speedup vs baseline: 1.2044x; 1.2044x over previous
"""Trainium2 Bass kernel for nn_BatchDropTop (topk row masking).

Reference math: per sample b, act = sum_c x[b,c,:,:]^2  -> [H,W]; L2-normalize
over flattened (H,W) (a positive per-sample scale -- cannot change any
ordering, so it is skipped); row score = max_w act -> [H]; drop (zero) the
rh=8 rows with the largest score; out = x * row_mask.

Kernel strategy (pure data parallel, batch 64 -> 8 samples on each of 8
cores; per core, per sample):
  - DMA x[s] (2048x24x8 f32, 1.5 MB) into SBUF as [128p, 16k, 192hw]
    (partition p holds channels 16p..16p+15; contiguous 12KB per partition).
  - ACT: square elementwise.
  - PE: 16 accumulating matmuls with a ones[128,1] stationary vector reduce
    the channel dim -> act [1, 192] in PSUM.
  - DVE: rowmax[1,24] = max over w; top8 = vector.max (8 largest, desc);
    mask[1,24] = (rowmax < top8[7]) as 1.0/0.0.  (Exactly the top-8 rows
    get 0; validated tie-free on the real inputs with 4.4e-5 min rel gap.)
  - DVE copy expands mask over w -> [1,192]; PE ones[1,128] matmul
    broadcasts it to [128,192] PSUM.
  - DVE: y = x * mask (mask AP broadcast over the 16 chunk dim), DMA out.

Everything is read from HBM once and written once: 25.2 MB per core
~= 70 us at the ~358 GB/s per-core HBM roofline; ACT/PE/DVE each have
~20-30 us of work, overlapped by the Tile scheduler.
"""

import sys

import numpy as np

for _p in ("/opt/trn_rl_repo", "/root/.axon_site/_ro/trn_rl_repo"):
    if _p not in sys.path:
        sys.path.append(_p)

B, C, H, W = 64, 2048, 24, 8
N_CORES = 8
BS = B // N_CORES  # samples per core
P = 128            # SBUF partitions
KC = C // P        # channel chunks per sample
HW = H * W
RH = 8             # rows to drop == round(0.33 * 24)

_cache = {}


def _build_nc():
    from concourse import bacc, mybir, tile

    f32 = mybir.dt.float32
    nc = bacc.Bacc("TRN2", target_bir_lowering=False, debug=False,
                   num_devices=N_CORES)
    x_in = nc.dram_tensor("x", [BS, C, H, W], f32, kind="ExternalInput")
    y_out = nc.dram_tensor("out", [BS, C, H, W], f32, kind="ExternalOutput")

    with tile.TileContext(nc) as tc:
        with (
            tc.tile_pool(name="xp", bufs=5) as xp,
            tc.tile_pool(name="sq", bufs=3) as sqp,
            tc.tile_pool(name="yp", bufs=3) as yp,
            tc.tile_pool(name="const", bufs=1) as constp,
            tc.tile_pool(name="small", bufs=BS) as smallp,
            tc.tile_pool(name="psA", bufs=3, space="PSUM") as psA,
            tc.tile_pool(name="psB", bufs=3, space="PSUM") as psB,
        ):
            ones_col = constp.tile([P, 1], f32)  # stationary K=128 reducer
            nc.vector.memset(ones_col[:], 1.0)
            ones_row = constp.tile([1, P], f32)  # stationary K=1 broadcaster
            nc.vector.memset(ones_row[:], 1.0)

            for s in range(BS):
                xt = xp.tile([P, KC, HW], f32, tag="x")
                nc.sync.dma_start(
                    out=xt[:],
                    in_=x_in[s].rearrange("(p k) h w -> p k (h w)", p=P),
                )

                xsq = sqp.tile([P, KC, HW], f32, tag="sq")
                nc.scalar.square(xsq[:], xt[:])

                act = psA.tile([1, HW], f32, tag="act")
                for k in range(KC):
                    nc.tensor.matmul(
                        act[:], ones_col[:], xsq[:, k, :],
                        start=(k == 0), stop=(k == KC - 1),
                    )

                rowmax = smallp.tile([1, H], f32, tag="rowmax")
                nc.vector.tensor_reduce(
                    rowmax[:],
                    act[:].rearrange("p (h w) -> p h w", h=H),
                    axis=mybir.AxisListType.X,
                    op=mybir.AluOpType.max,
                )
                top8 = smallp.tile([1, RH], f32, tag="top8")
                nc.vector.max(top8[:], rowmax[:])
                maskh = smallp.tile([1, H], f32, tag="maskh")
                nc.vector.tensor_single_scalar(
                    maskh[:], rowmax[:], top8[0:1, RH - 1:RH],
                    mybir.AluOpType.is_lt,
                )
                maskhw = smallp.tile([1, HW], f32, tag="maskhw")
                nc.vector.tensor_copy(
                    maskhw[:].rearrange("p (h w) -> p h w", h=H),
                    maskh[:].unsqueeze(2).broadcast_to([1, H, W]),
                )

                mb = psB.tile([P, HW], f32, tag="mb")
                nc.tensor.matmul(mb[:], ones_row[:], maskhw[:],
                                 start=True, stop=True)
                mbs = smallp.tile([P, HW], f32, tag="mbs")
                nc.scalar.copy(mbs[:], mb[:])

                yt = yp.tile([P, KC, HW], f32, tag="y")
                nc.vector.tensor_tensor(
                    yt[:], xt[:],
                    mbs[:].unsqueeze(1).broadcast_to([P, KC, HW]),
                    op=mybir.AluOpType.mult,
                )
                # Stores ride gpsimd's HWDGE ring so they never FIFO-block
                # the loads issued on sync's ring.
                nc.gpsimd.dma_start(
                    out=y_out[s].rearrange("(p k) h w -> p k (h w)", p=P),
                    in_=yt[:],
                )

    nc.compile()
    return nc


def get_nc():
    if "nc" not in _cache:
        _cache["nc"] = _build_nc()
    return _cache["nc"]


def kernel(x):
    from concourse.bass_utils import run_bass_kernel_spmd

    x = np.ascontiguousarray(np.asarray(x, dtype=np.float32))
    assert x.shape == (B, C, H, W), x.shape
    nc = get_nc()
    in_maps = [{"x": x[i * BS:(i + 1) * BS]} for i in range(N_CORES)]
    res = run_bass_kernel_spmd(nc, in_maps, list(range(N_CORES)))
    return np.concatenate(
        [res.results[i]["out"] for i in range(N_CORES)], axis=0
    )
